# revision 48
# baseline (speedup 1.0000x reference)
"""Trainium2 Bass kernel for nn_MessagePassingBlock (GNN message passing).

Math (reference):
    h     = x @ W_msg                       # (N, D)
    msg   = (h[source] + rel_bias[edge_type]) * edge_weights[:, None]
    delta = segment_sum(msg, target, N)     # (N, D)
    out   = relu(x @ W_self + delta + b)

Rewritten per target block B of 128 nodes (w_e folded into the gathered
x rows, rel_bias/bias folded into a host bincount CT9):
    acc[k, j]  = sum_e (w_e x[s_e, k]) * [tgt_e == j]     (chunked PE matmuls)
    out_B^T    = relu(W_msg^T @ acc + W_self^T @ x_B^T + rb9^T @ CT9_B)
where CT9_B[r, j] = sum_{e->j} w_e [et_e == r], row 8 = ones carrying b.

Distribution: target-sharded across 8 cores, no collectives. Global target
blocks (128 nodes each) are snake-dealt to cores by edge count so every
core's slot s holds a similar-count block (minimizes static chunk padding).

Per 128-edge chunk the device does ONE matmul:
    lhsT = wx chunk [e, k] bf16   (host-reordered w_e*x[s_e] rows,
                                   contiguous HWDGE stream - no gather)
    rhs  = one-hot  [e, j]        (target one-hot; 0/1 exact)
accumulating acc into a per-group PSUM tile; per half-group a 3-matmul
epilogue (W_msg / W_self / rel+bias) and one ACT relu produce the output.

The one-hot supply is split to balance HBM vs DVE: for SPLIT blocks of
each 7-block group it is streamed from the host as fp8; for the rest it
is built on-chip with one DVE tensor_scalar is_equal per chunk
(iota == tgt, 2x perf mode; the edge weight already lives in the wx
rows, keeping the on-chip build a single-op compare).  The PSUM->bf16
cast rides the Scalar engine (ACT Copy) and the output writeback rides
the idle SWDGE ring, so the two HWDGE rings carry nothing but prefetch.
"""

import numpy as np
import ml_dtypes

NUM_NODES = 100000
D = 128
NUM_REL = 8
N_CORES = 8
NBLK = 98                      # blocks (slots) per core
NODES_PER_CORE = NBLK * 128    # 12544
GBLK = N_CORES * NBLK          # 784 global blocks, 100352 padded nodes
GRP = 7                        # blocks per processing group
N_GRP = NBLK // GRP            # 14
HGRP = GRP * 128 // 2          # 448 cols per epilogue half

_kernel_cache = {}


def _build_and_compile(c_s):
    """Build + compile the SPMD Bass kernel for static per-slot chunk
    capacities c_s (tuple of NBLK ints, all >= 1)."""
    import concourse.bacc as bacc
    import concourse.tile as tile
    import concourse.mybir as mybir

    c_s = list(c_s)
    off = np.concatenate([[0], np.cumsum(c_s)]).astype(int)  # chunk offsets
    NC_TOT = int(off[-1])
    nck_g = [int(off[(g + 1) * GRP] - off[g * GRP]) for g in range(N_GRP)]
    nckmax = max(nck_g)
    nckmax_b = max(
        int(off[(g + 1) * GRP] - off[g * GRP + 2]) for g in range(N_GRP)
    )  # DVE-built one-hot chunks per group (blocks SPLIT..GRP)

    nc = bacc.Bacc(
        "TRN2",
        target_bir_lowering=False,
        debug=False,
        num_devices=N_CORES,
    )
    f32 = mybir.dt.float32
    bf16 = mybir.dt.bfloat16
    fp8 = mybir.dt.float8e4
    RELU = mybir.ActivationFunctionType.Relu
    EQ = mybir.AluOpType.is_equal
    SPLIT = 2          # blocks/group with host-streamed one-hot; rest DVE-built

    hg_d = nc.dram_tensor("hg", [128, NC_TOT * 128], bf16, kind="ExternalInput")
    oh_d = nc.dram_tensor("oh", [128, NC_TOT * 128], fp8, kind="ExternalInput")
    xt_d = nc.dram_tensor("xt", [128, NODES_PER_CORE], fp8, kind="ExternalInput")
    ct9_d = nc.dram_tensor("ct9", [NUM_REL + 1, NODES_PER_CORE], bf16, kind="ExternalInput")
    wmsg_d = nc.dram_tensor("wmsg", [D, D], bf16, kind="ExternalInput")
    wself_d = nc.dram_tensor("wself", [D, D], bf16, kind="ExternalInput")
    rb9_d = nc.dram_tensor("rb9", [NUM_REL + 1, D], bf16, kind="ExternalInput")
    tgtm_d = nc.dram_tensor("tgtm", [128, NC_TOT], f32, kind="ExternalInput")
    iota_d = nc.dram_tensor("iota", [128, 128], bf16, kind="ExternalInput")
    out_d = nc.dram_tensor("out", [D, NODES_PER_CORE], bf16, kind="ExternalOutput")

    with tile.TileContext(nc) as tc:
        with tc.tile_pool(name="const", bufs=1) as cpool, tc.tile_pool(
            name="hgp", bufs=4
        ) as hgpool, tc.tile_pool(name="ohp", bufs=3) as ohpool, tc.tile_pool(
            name="xtp", bufs=3
        ) as xtpool, tc.tile_pool(name="ohbp", bufs=2) as ohbpool, tc.tile_pool(
            name="o7p", bufs=3
        ) as o7pool, tc.tile_pool(
            name="arp", bufs=3
        ) as arpool, tc.tile_pool(name="psA", bufs=3, space="PSUM") as psA, tc.tile_pool(
            name="psO", bufs=2, space="PSUM"
        ) as psO:
            # ---- constants (one DMA each) ----
            # iota/tgtm gate the very first DVE one-hot builds: put them at
            # the head of the sync ring (tiny, ~1us) so Vector ramps early
            tgtm_t = cpool.tile([128, NC_TOT], f32)
            nc.sync.dma_start(out=tgtm_t[:], in_=tgtm_d.ap())
            iota_t = cpool.tile([128, 128], bf16)
            nc.sync.dma_start(out=iota_t[:], in_=iota_d.ap())
            # remaining consts ride the idle SWDGE ring so the HWDGE rings
            # carry the group-0 streams immediately
            wmsg_t = cpool.tile([D, D], bf16)
            nc.gpsimd.dma_start(out=wmsg_t[:], in_=wmsg_d.ap())
            wself_t = cpool.tile([D, D], bf16)
            nc.gpsimd.dma_start(out=wself_t[:], in_=wself_d.ap())
            rb9_t = cpool.tile([NUM_REL + 1, D], bf16)
            nc.gpsimd.dma_start(out=rb9_t[:], in_=rb9_d.ap())
            ct9_t = cpool.tile([NUM_REL + 1, NODES_PER_CORE], bf16)
            nc.gpsimd.dma_start(out=ct9_t[:], in_=ct9_d.ap())

            def do_group(g, hg_t, oh_t, ohb_t, xt_t):
                """Chunk matmuls for group g; returns psA group tile.
                Blocks [0, SPLIT) read the host-streamed fp8 one-hot oh_t;
                blocks [SPLIT, GRP) read the DVE-built bf16 one-hot ohb_t."""
                s0 = g * GRP
                bsplit = int(off[s0 + SPLIT] - off[s0])
                # DVE one-hot builds for the tail blocks, issued up front
                for bi in range(SPLIT, GRP):
                    s = s0 + bi
                    coff = int(off[s] - off[s0])
                    for ci in range(int(c_s[s])):
                        k = coff + ci
                        ka = int(off[s]) + ci
                        nc.vector.tensor_scalar(
                            out=ohb_t[:, (k - bsplit) * 128 : (k - bsplit + 1) * 128],
                            in0=iota_t[:],
                            scalar1=tgtm_t[:, ka : ka + 1],
                            scalar2=None,
                            op0=EQ,
                        )
                psA_t = psA.tile([128, GRP * 128], f32, tag="acc")
                for bi in range(GRP):
                    s = s0 + bi
                    cs = int(c_s[s])
                    coff = int(off[s] - off[s0])
                    for ci in range(cs):
                        k = coff + ci
                        if bi < SPLIT:
                            rhs = oh_t[:, k * 128 : (k + 1) * 128]
                        else:
                            rhs = ohb_t[:, (k - bsplit) * 128 : (k - bsplit + 1) * 128]
                        nc.tensor.matmul(
                            out=psA_t[:, bi * 128 : (bi + 1) * 128],
                            lhsT=hg_t[:, k * 128 : (k + 1) * 128],
                            rhs=rhs,
                            start=(ci == 0),
                            stop=(ci == cs - 1),
                        )
                return psA_t

            def do_epilogue(g, psA_t, xt_t):
                """Cast + 3-matmul epilogue + relu + out DMA for group g."""
                s0 = g * GRP
                araw = arpool.tile([128, GRP * 128], bf16, tag="araw")
                # cast on ACT (near PSUM, lightly loaded) to keep DVE free
                nc.scalar.activation(
                    out=araw[:], in_=psA_t[:],
                    func=mybir.ActivationFunctionType.Copy,
                )
                o7_t = o7pool.tile([128, GRP * 128], bf16, tag="o7")
                for h in range(2):
                    c0 = h * HGRP
                    psO_t = psO.tile([128, HGRP], f32, tag="o2")
                    nc.tensor.matmul(
                        out=psO_t[:], lhsT=wmsg_t[:],
                        rhs=araw[:, c0 : c0 + HGRP],
                        start=True, stop=False,
                    )
                    nc.tensor.matmul(
                        out=psO_t[:], lhsT=wself_t[:],
                        rhs=xt_t[:, c0 : c0 + HGRP],
                        start=False, stop=False,
                    )
                    nc.tensor.matmul(
                        out=psO_t[:], lhsT=rb9_t[:],
                        rhs=ct9_t[:, s0 * 128 + c0 : s0 * 128 + c0 + HGRP],
                        start=False, stop=True,
                    )
                    nc.scalar.activation(
                        out=o7_t[:, c0 : c0 + HGRP], in_=psO_t[:], func=RELU
                    )
                # writeback on the scalar (ACT) ring: it is the only DMA
                # class there and issues right after the relu it depends on,
                # avoiding SWDGE's ~2us fixed latency in the tail
                nc.scalar.dma_start(
                    out=out_d.ap()[:, s0 * 128 : (s0 + GRP) * 128], in_=o7_t[:]
                )

            pend = None  # (g, psA_t, xt_t) pending epilogue
            for g in range(N_GRP):
                s0 = g * GRP
                ng = nck_g[g]
                hg_t = hgpool.tile([128, nckmax * 128], bf16, tag="hg")
                oh_t = ohpool.tile([128, nckmax * 128], fp8, tag="oh")
                ohb_t = ohbpool.tile([128, nckmax_b * 128], bf16, tag="ohb")
                # load in pieces for finer pipelining / faster rampup
                for lo, hi in ((0, 2), (2, 4), (4, GRP)):
                    p0 = int(off[s0 + lo] - off[s0])
                    p1 = int(off[s0 + hi] - off[s0])
                    nc.sync.dma_start(
                        out=hg_t[:, p0 * 128 : p1 * 128],
                        in_=hg_d.ap()[
                            :, int(off[s0 + lo]) * 128 : int(off[s0 + hi]) * 128
                        ],
                    )
                    if lo < SPLIT:  # one-hot streamed only for blocks < SPLIT
                        # sync (SP) ring too: scalar-ring DMAs issue from the
                        # ACT queue, where a cast/relu waiting on PSUM would
                        # stall the prefetch issue behind it
                        q0, q1 = p0, min(p1, int(off[s0 + SPLIT] - off[s0]))
                        nc.sync.dma_start(
                            out=oh_t[:, q0 * 128 : q1 * 128],
                            in_=oh_d.ap()[
                                :,
                                (int(off[s0]) + q0) * 128 : (int(off[s0]) + q1) * 128,
                            ],
                        )
                xt_t = xtpool.tile([128, GRP * 128], fp8, tag="xt")
                nc.sync.dma_start(
                    out=xt_t[:], in_=xt_d.ap()[:, s0 * 128 : (s0 + GRP) * 128]
                )
                psA_t = do_group(g, hg_t, oh_t, ohb_t, xt_t)
                if pend is not None:
                    do_epilogue(*pend)
                pend = (g, psA_t, xt_t)
            do_epilogue(*pend)

    nc.compile()
    return nc


def _prep(inputs):
    """Host-side sharding/layout. Returns (in_maps, static_key, layout)."""
    x = np.ascontiguousarray(np.asarray(inputs["x"], dtype=np.float32))
    source = np.asarray(inputs["source"]).astype(np.int64)
    target = np.asarray(inputs["target"]).astype(np.int64)
    edge_type = np.asarray(inputs["edge_type"]).astype(np.int64)
    ew = np.asarray(inputs["edge_weights"], dtype=np.float32)
    w_msg = np.asarray(inputs["W_msg"], dtype=np.float32)
    rel_bias = np.asarray(inputs["rel_bias"], dtype=np.float32)
    w_self = np.asarray(inputs["W_self"], dtype=np.float32)
    b = np.asarray(inputs["b"], dtype=np.float32).reshape(D)

    assert x.shape[0] == NUM_NODES

    # ---- snake-deal global blocks to cores by edge count ----
    gb_of_edge = target >> 7                      # global block per edge
    cntg = np.bincount(gb_of_edge, minlength=GBLK)
    order_desc = np.argsort(-cntg, kind="stable")  # blocks by count desc
    core_of_gb = np.empty(GBLK, dtype=np.int64)
    slot_of_gb = np.empty(GBLK, dtype=np.int64)
    for s in range(NBLK):
        chunk = order_desc[s * N_CORES : (s + 1) * N_CORES]
        cores = np.arange(N_CORES) if s % 2 == 0 else np.arange(N_CORES)[::-1]
        core_of_gb[chunk] = cores
        slot_of_gb[chunk] = s
    gb_at = np.empty((N_CORES, NBLK), dtype=np.int64)
    gb_at[core_of_gb, slot_of_gb] = np.arange(GBLK)

    core_e = core_of_gb[gb_of_edge]
    slot_e = slot_of_gb[gb_of_edge]
    tib_e = target & 127                           # target-in-block

    # ---- static chunk capacities per slot ----
    cnt_cs = np.zeros((N_CORES, NBLK), dtype=np.int64)
    np.add.at(cnt_cs, (core_e, slot_e), 1)
    c_s = np.maximum(np.ceil(cnt_cs.max(axis=0) / 128).astype(np.int64), 1)
    off = np.concatenate([[0], np.cumsum(c_s)]).astype(np.int64)
    NC_TOT = int(off[-1])
    S = NC_TOT * 128

    in_maps = []
    wmsg_bf = np.ascontiguousarray(w_msg.astype(ml_dtypes.bfloat16))
    wself_bf = np.ascontiguousarray(w_self.astype(ml_dtypes.bfloat16))
    rb9 = np.concatenate([rel_bias, b.reshape(1, D)], axis=0)
    rb9_bf = np.ascontiguousarray(rb9.astype(ml_dtypes.bfloat16))
    iota_bf = np.ascontiguousarray(
        np.broadcast_to(np.arange(128, dtype=np.float32), (128, 128)).astype(
            ml_dtypes.bfloat16
        )
    )

    for c in range(N_CORES):
        m = core_e == c
        e_slot = slot_e[m]
        e_src = source[m]
        e_tib = tib_e[m]
        e_w = ew[m]
        e_et = edge_type[m]
        ordr = np.argsort(e_slot, kind="stable")
        eb = e_slot[ordr]
        cnt_c = np.bincount(eb, minlength=NBLK)
        starts = np.concatenate([[0], np.cumsum(cnt_c)[:-1]])
        pos = np.arange(len(eb)) - starts[eb]
        slots = off[eb] * 128 + pos

        src_slot = np.zeros(S, dtype=np.int64)
        src_slot[slots] = e_src[ordr]
        tgt_slot = np.zeros(S, dtype=np.int64)
        tgt_slot[slots] = e_tib[ordr]
        w_slot = np.zeros(S, dtype=np.float32)
        w_slot[slots] = e_w[ordr]
        tgtm = np.ascontiguousarray(
            tgt_slot.reshape(NC_TOT, 128).T.astype(np.float32)
        )

        # weighted gathered rows, chunk-tile layout [128, NC_TOT*128]
        wx = (x[src_slot] * w_slot[:, None]).astype(ml_dtypes.bfloat16)
        hg = np.ascontiguousarray(
            wx.reshape(NC_TOT, 128, 128).transpose(1, 0, 2).reshape(128, -1)
        )
        # unweighted target one-hot, fp8 (0/1 exact)
        oh = np.zeros((S, 128), dtype=ml_dtypes.float8_e4m3)
        oh[np.arange(S), tgt_slot] = 1.0
        oh = np.ascontiguousarray(
            oh.reshape(NC_TOT, 128, 128).transpose(1, 0, 2).reshape(128, -1)
        )

        # node-major x shard (pad rows zero), then feature-major bf16
        glob = (gb_at[c][:, None] * 128 + np.arange(128)[None, :]).reshape(-1)
        valid = glob < NUM_NODES
        xsh = np.zeros((NODES_PER_CORE, D), dtype=np.float32)
        xsh[valid] = x[glob[valid]]
        xt = np.ascontiguousarray(xsh.T.astype(ml_dtypes.float8_e4m3))

        # per-target weighted relation counts (+ ones row for bias b)
        loc = e_slot * 128 + e_tib
        idx = loc * NUM_REL + e_et
        ct8 = np.bincount(
            idx, weights=e_w, minlength=NODES_PER_CORE * NUM_REL
        ).reshape(NODES_PER_CORE, NUM_REL)
        ct9 = np.concatenate(
            [ct8.T, np.ones((1, NODES_PER_CORE))], axis=0
        ).astype(ml_dtypes.bfloat16)
        ct9 = np.ascontiguousarray(ct9)

        in_maps.append(
            {
                "hg": hg,
                "oh": oh,
                "xt": xt,
                "ct9": ct9,
                "wmsg": wmsg_bf,
                "wself": wself_bf,
                "rb9": rb9_bf,
                "tgtm": tgtm,
                "iota": iota_bf,
            }
        )

    static_key = tuple(c_s.tolist())
    return in_maps, static_key, (gb_at,)


def kernel(**inputs) -> np.ndarray:
    from concourse import bass_utils

    in_maps, static_key, (gb_at,) = _prep(inputs)

    nc = _kernel_cache.get(static_key)
    if nc is None:
        nc = _build_and_compile(static_key)
        _kernel_cache[static_key] = nc

    res = bass_utils.run_bass_kernel_spmd(
        nc, in_maps, core_ids=list(range(N_CORES))
    )
    full = np.zeros((NUM_NODES, D), dtype=np.float32)
    for c in range(N_CORES):
        outc = res.results[c]["out"].astype(np.float32).T  # [12544, 128]
        glob = (gb_at[c][:, None] * 128 + np.arange(128)[None, :]).reshape(-1)
        valid = glob < NUM_NODES
        full[glob[valid]] = outc[valid]
    return np.ascontiguousarray(full)
